# revision 14
# baseline (speedup 1.0000x reference)
"""Trainium2 Bass kernel for DkNetCL (4x [3x3 conv + SRePro] + FC 32768->10).

Strategy (pure data parallel over 8 cores, 128 images/core):
- Images processed in groups of 4, stacked on SBUF partitions as
  [4 img x 32 ch] = 128 partitions. Convs become block-diagonal matmuls
  (lhsT [128,128] block-diag per 3x3 tap), with the 9 taps realized as
  shifted access patterns over zero-padded 34x34 images laid out in the
  free dimension (guard zones make every shifted read safe).
- Layer 0 (3->32 ch): im2col done by the DMA engines - x is read 9 times
  from HBM, each tap written pixel-shifted to partitions 32j+3t+c, so the
  whole first conv is 3 matmuls per group (K=27 used rows per block).
- SRePro per group: ACT square+accum -> ones-block-diag matmul reduces
  over channels within each image block (replicating the norm across the
  block) -> DVE computes 1/(1+s/2) -> fused scale+copy PSUM->SBUF.
- FC: last layer's scaled output goes to a compact [128, 1024] tile per
  group, PE-transposes to [pix, img*ch], then 256 accumulating matmuls
  (lhsT = actT [128pix, img], rhs = fc_w tile [128pix, 10]) produce
  y[img, 10] directly. Bias added on DVE.
- Matmuls use float32r (full-rate fp32 mode on the PE) by default.
"""

import numpy as np

CORES = 8
B_PER_CORE = 128
SUB = 32            # images per sub-batch
NGRP = SUB // 4     # groups of 4 images
NSUB = B_PER_CORE // SUB
S = 1191            # per-image span in act buffers (1156 + 35)
GUARD = 35
CHUNKS = [(0, 12), (12, 24), (24, 34)]   # bank-aligned row chunks
TAPS = [(dy, dx) for dy in (-1, 0, 1) for dx in (-1, 0, 1)]

MM_DT = "float32"   # dtype used for conv/fc matmul operands


def build_bass():
    import concourse.bass as bass
    import concourse.mybir as mybir
    import concourse.tile as tile
    from concourse import bacc

    f32 = mybir.dt.float32
    mmdt = getattr(mybir.dt, MM_DT)
    AF = mybir.ActivationFunctionType
    ALU = mybir.AluOpType

    nc = bacc.Bacc("TRN2", target_bir_lowering=False, debug=False)

    x_d = nc.dram_tensor("x", [B_PER_CORE, 3, 32, 32], f32, kind="ExternalInput")
    w0_d = nc.dram_tensor("conv_w0", [32, 3, 3, 3], f32, kind="ExternalInput")
    w_d = [w0_d] + [
        nc.dram_tensor(f"conv_w{i}", [32, 32, 3, 3], f32, kind="ExternalInput")
        for i in (1, 2, 3)
    ]
    fcw_d = nc.dram_tensor("fc_w", [10, 32768], f32, kind="ExternalInput")
    fcb_d = nc.dram_tensor("fc_b", [10], f32, kind="ExternalInput")
    y_d = nc.dram_tensor("y", [B_PER_CORE, 10], f32, kind="ExternalOutput")

    def mm(ap):
        return ap.bitcast(mmdt) if MM_DT != "float32" else ap

    with tile.TileContext(nc) as tc:
        with (
            tc.tile_pool(name="persist", bufs=1) as pp,
            tc.tile_pool(name="work", bufs=2) as wp,
            tc.tile_pool(name="cpsum", bufs=2, space="PSUM") as cpool,
            tc.tile_pool(name="spsum", bufs=2, space="PSUM") as spool,
        ):
            # ---------- persistent tiles ----------
            x_sb = pp.tile([128, NGRP * 1156], f32, tag="x_sb")
            actA = pp.tile([128, GUARD + NGRP * S], f32, tag="actA")
            actB = pp.tile([128, GUARD + NGRP * S], f32, tag="actB")
            actT = pp.tile([128, 8 * 32 * SUB], f32, tag="actT")  # [c8][ch][img]
            fc_sb = pp.tile([128, 32 * 8 * 10], f32, tag="fc_sb")  # [ch][c8][o]
            bias_sb = pp.tile([SUB, 10], f32, tag="bias")
            w0_bd = pp.tile([128, 128], f32, tag="w0bd")
            wbd = {(L, t): pp.tile([128, 128], f32, tag=f"w{L}_{t}",
                                   name=f"w{L}_{t}")
                   for L in (1, 2, 3) for t in range(9)}
            ones_bd = pp.tile([128, 128], f32, tag="ones")
            ident = pp.tile([128, 128], f32, tag="ident")
            iota_a = pp.tile([128, 128], mybir.dt.int32, tag="iota_a")
            iota_b = pp.tile([128, 128], mybir.dt.int32, tag="iota_b")

            # ---------- init: zeros, weights, identity ----------
            nc.vector.memset(x_sb[:], 0.0)
            nc.vector.memset(actA[:], 0.0)
            nc.vector.memset(actB[:], 0.0)
            nc.vector.memset(w0_bd[:], 0.0)
            nc.vector.memset(ones_bd[:], 0.0)
            for j in range(4):
                nc.vector.memset(ones_bd[32*j:32*j+32, 32*j:32*j+32], 1.0)

            # identity: iota col (per-row 0..127), iota row (partition idx)
            nc.gpsimd.iota(iota_a[:], pattern=[[1, 128]], base=0,
                           channel_multiplier=0)
            nc.gpsimd.iota(iota_b[:], pattern=[[0, 128]], base=0,
                           channel_multiplier=1)
            nc.vector.tensor_tensor(ident[:], iota_a[:], iota_b[:],
                                    ALU.is_equal)

            # conv_w0 -> w0_bd[32j + 3t + c, 32j + o]
            # dram conv_w0 [o, c, dy, dx]; per j one DMA:
            # out dims [t 9 (p-step 3), c 3 (p-step 1)][free o 32]
            w0src = w0_d[:].rearrange("o c dy dx -> (dy dx) c o")
            for j in range(4):
                for t in range(9):
                    nc.sync.dma_start(
                        out=w0_bd[32*j+3*t:32*j+3*t+3, 32*j:32*j+32],
                        in_=w0src[t])
            # conv_w{1..3} -> wbd[(L,t)][32j+ci, 32j+co] = w[co, ci, dy, dx]
            for L in (1, 2, 3):
                wsrc = w_d[L][:].rearrange("co ci dy dx -> (dy dx) ci co")
                for t in range(9):
                    nc.vector.memset(wbd[(L, t)][:], 0.0)
                    for j in range(4):
                        nc.sync.dma_start(
                            out=wbd[(L, t)][32*j:32*j+32, 32*j:32*j+32],
                            in_=wsrc[t])

            # fc_w -> fc_sb[p, (ch, c8, o)] = fc_w[o, ch*1024 + c8*128 + p]
            fcv = fc_sb[:].rearrange("p (ch c8 o) -> p ch c8 o", ch=32, c8=8)
            fsrc = fcw_d[:].rearrange("o (ch c8 p) -> ch c8 p o", ch=32, c8=8)
            for ch in range(32):
                for c8 in range(8):
                    nc.sync.dma_start(out=fcv[:, ch, c8], in_=fsrc[ch, c8])
            # bias replicated across SUB partitions
            for i in range(SUB):
                nc.sync.dma_start(out=bias_sb[i:i+1, :], in_=fcb_d[None, :])

            # ---------- main loop ----------
            xr = x_d[:].rearrange("(s g il) c h w -> s il g c h w",
                                  s=NSUB, g=NGRP)
            x_sbv = x_sb[:].rearrange("(il pc) (g q) -> il pc g q",
                                      il=4, g=NGRP)

            for s in range(NSUB):
                # L0 im2col: 27 DMAs, each tap written pixel-shifted
                xq = x_sbv.rearrange("il pc g (yy xx) -> il pc g yy xx", yy=34)
                for t, (dy, dx) in enumerate(TAPS):
                    for c in range(3):
                        for g in range(NGRP):
                            nc.sync.dma_start(
                                out=xq[:, 3*t + c, g,
                                       1-dy:33-dy, 1-dx:33-dx],
                                in_=xr[s, :, g, c, :, :])

                bufs = [actA, actB]
                aTv = actT[:].rearrange("p (c8 ch i) -> p c8 ch i",
                                        c8=8, ch=32)
                for L in range(4):
                    dst = bufs[L % 2]
                    src = bufs[(L - 1) % 2]
                    for g in range(NGRP):
                        psum = cpool.tile([128, 1536], f32, tag="conv")
                        for ci, (r0, r1) in enumerate(CHUNKS):
                            n = (r1 - r0) * 34
                            po = psum[:, 512*ci: 512*ci + n]
                            if L == 0:
                                rhs = x_sb[:, g*1156 + r0*34:
                                           g*1156 + r0*34 + n]
                                nc.tensor.matmul(po, mm(w0_bd[:]), mm(rhs),
                                                 start=True, stop=True)
                            else:
                                for t in range(9):
                                    dy, dx = TAPS[t]
                                    o = GUARD + g*S + r0*34 + dy*34 + dx
                                    rhs = src[:, o:o+n]
                                    nc.tensor.matmul(
                                        po, mm(wbd[(L, t)][:]), mm(rhs),
                                        start=(t == 0), stop=(t == 8))

                        # chunk ci as [p, y, x] rows r0..r1 of 34 pixels
                        def chunk_rows(ci):
                            r0, r1 = CHUNKS[ci]
                            return psum[:, 512*ci: 512*ci + (r1-r0)*34] \
                                .rearrange("p (y x) -> p y x", x=34)

                        # ---- SRePro ----
                        sq = wp.tile([128, 3], f32, tag="sq")
                        scratch = wp.tile([128, 408], f32, tag="scr")
                        for ci, (r0, r1) in enumerate(CHUNKS):
                            y0, y1 = max(r0, 1), min(r1, 33)
                            iv = chunk_rows(ci)[:, y0-r0:y1-r0, 1:33]
                            nc.scalar.activation(
                                scratch[:, :(y1-y0)*32].rearrange(
                                    "p (y x) -> p y x", x=32),
                                iv, AF.Square, accum_out=sq[:, ci:ci+1])
                        sqt = wp.tile([128, 1], f32, tag="sqt")
                        nc.vector.tensor_reduce(
                            sqt[:], sq[:], axis=mybir.AxisListType.X,
                            op=ALU.add)
                        nrm = spool.tile([128, 1], f32, tag="small")
                        nc.tensor.matmul(nrm[:], ones_bd[:], sqt[:],
                                         start=True, stop=True)
                        scl = wp.tile([128, 1], f32, tag="scl")
                        scl2 = wp.tile([128, 1], f32, tag="scl2")
                        nc.vector.tensor_scalar(scl[:], nrm[:], 0.5, 1.0,
                                                ALU.mult, ALU.add)
                        nc.vector.reciprocal(scl2[:], scl[:])

                        # ---- scaled copy out of PSUM ----
                        if L < 3:
                            base = GUARD + g*S
                            dv = dst[:, base: base + 1156].rearrange(
                                "p (y x) -> p y x", x=34)
                            for ci, (r0, r1) in enumerate(CHUNKS):
                                y0, y1 = max(r0, 1), min(r1, 33)
                                iv = chunk_rows(ci)[:, y0-r0:y1-r0, 1:33]
                                nc.vector.tensor_scalar(
                                    dv[:, y0:y1, 1:33], iv, scl2[:],
                                    None, ALU.mult)
                        else:
                            comp = wp.tile([128, 1024], f32, tag="comp")
                            cv = comp[:].rearrange("p (y x) -> p y x", x=32)
                            for ci, (r0, r1) in enumerate(CHUNKS):
                                y0, y1 = max(r0, 1), min(r1, 33)
                                iv = chunk_rows(ci)[:, y0-r0:y1-r0, 1:33]
                                nc.vector.tensor_scalar(
                                    cv[:, y0-1:y1-1, :], iv, scl2[:],
                                    None, ALU.mult)
                            # transpose compact -> actT right away
                            for c8 in range(8):
                                pT = spool.tile([128, 128], f32, tag="small")
                                nc.tensor.transpose(
                                    pT[:], comp[:, 128*c8:128*(c8+1)],
                                    ident[:])
                                # pT[pix, il*32+ch] -> actT[pix,c8,ch,g4+il]
                                nc.vector.tensor_copy(
                                    aTv[:, c8, :, 4*g:4*g+4],
                                    pT[:].rearrange("p (il ch) -> p ch il",
                                                    il=4))

                # ---- FC ----
                yp = spool.tile([SUB, 10], f32, tag="small")
                nmm = 0
                for c8 in range(8):
                    for ch in range(32):
                        nc.tensor.matmul(
                            yp[:], mm(aTv[:, c8, ch, :]),
                            mm(fcv[:, ch, c8, :]),
                            start=(nmm == 0), stop=(nmm == 255))
                        nmm += 1
                y_sb = wp.tile([SUB, 10], f32, tag="ysb")
                nc.vector.tensor_tensor(y_sb[:], yp[:], bias_sb[:], ALU.add)
                nc.sync.dma_start(out=y_d[SUB*s:SUB*(s+1), :], in_=y_sb[:])

    nc.compile()
    return nc


_NC_CACHE = None


def kernel(**inputs):
    global _NC_CACHE
    from concourse.bass_utils import run_bass_kernel_spmd

    if _NC_CACHE is None:
        _NC_CACHE = build_bass()
    nc = _NC_CACHE

    x = np.ascontiguousarray(inputs["x"], np.float32)
    shared = {k: np.ascontiguousarray(np.asarray(inputs[k]), np.float32)
              for k in ("conv_w0", "conv_w1", "conv_w2", "conv_w3",
                        "fc_w", "fc_b")}
    in_maps = [
        {"x": x[i*B_PER_CORE:(i+1)*B_PER_CORE], **shared} for i in range(CORES)
    ]
    res = run_bass_kernel_spmd(nc, in_maps, core_ids=list(range(CORES)))
    return np.concatenate([r["y"] for r in res.results], axis=0)


# revision 19
# speedup vs baseline: 101.6311x; 101.6311x over previous
"""Trainium2 Bass kernel for DkNetCL (4x [3x3 conv + SRePro] + FC 32768->10).

Strategy (pure data parallel over 8 cores, 128 images/core):
- Images processed in groups of 4, stacked on SBUF partitions as
  [4 img x 32 ch] = 128 partitions. Convs become block-diagonal matmuls
  (lhsT [128,128] block-diag per 3x3 tap), with the 9 taps realized as
  shifted access patterns over zero-padded 34x34 images laid out in the
  free dimension (guard zones make every shifted read safe).
- Layer 0 (3->32 ch): im2col done by the DMA engines - x is read 9 times
  from HBM, each tap written pixel-shifted to partitions 32j+3t+c, so the
  whole first conv is 3 matmuls per group (K=27 used rows per block).
- SRePro per group: ACT square+accum -> ones-block-diag matmul reduces
  over channels within each image block (replicating the norm across the
  block) -> DVE computes 1/(1+s/2) -> fused scale+copy PSUM->SBUF.
- FC: last layer's scaled output goes to a compact [128, 1024] tile per
  group, PE-transposes to [pix, img*ch], then 256 accumulating matmuls
  (lhsT = actT [128pix, img], rhs = fc_w tile [128pix, 10]) produce
  y[img, 10] directly. Bias added on DVE.
- Matmuls use float32r: fp32 bits in SBUF, full-rate (1 col/cycle) on
  the PE vs 4 cycles/col for strict fp32. Measured end-to-end rel err
  vs the fp32 reference: ~3e-4 (tf32-like rounding in the PE).
"""

import numpy as np

CORES = 8
B_PER_CORE = 128
SUB = 32            # images per sub-batch
NGRP = SUB // 4     # groups of 4 images
NSUB = B_PER_CORE // SUB
S = 1191            # per-image span in act buffers (1156 + 35)
GUARD = 35
CHUNKS = [(0, 12), (12, 24), (24, 34)]   # bank-aligned row chunks
TAPS = [(dy, dx) for dy in (-1, 0, 1) for dx in (-1, 0, 1)]

MM_DT = "float32r"  # dtype used for conv/fc matmul operands
REPEAT = 0          # >0: wrap main loop in a hardware For_i for timing


def build_bass():
    import concourse.bass as bass
    import concourse.mybir as mybir
    import concourse.tile as tile
    from concourse import bacc

    f32 = mybir.dt.float32
    mdt = getattr(mybir.dt, MM_DT)
    AF = mybir.ActivationFunctionType
    ALU = mybir.AluOpType

    nc = bacc.Bacc("TRN2", target_bir_lowering=False, debug=False)

    x_d = nc.dram_tensor("x", [B_PER_CORE, 3, 32, 32], f32, kind="ExternalInput")
    w0_d = nc.dram_tensor("conv_w0", [32, 3, 3, 3], f32, kind="ExternalInput")
    w_d = [w0_d] + [
        nc.dram_tensor(f"conv_w{i}", [32, 32, 3, 3], f32, kind="ExternalInput")
        for i in (1, 2, 3)
    ]
    fcw_d = nc.dram_tensor("fc_w", [10, 32768], f32, kind="ExternalInput")
    fcb_d = nc.dram_tensor("fc_b", [10], f32, kind="ExternalInput")
    y_d = nc.dram_tensor("y", [B_PER_CORE, 10], f32, kind="ExternalOutput")

    def dmt(ap):
        # bitcast a DRAM f32 AP for DMA into an mdt tile (bit-identical)
        return ap.bitcast(mdt) if MM_DT != "float32" else ap

    with tile.TileContext(nc) as tc:
        with (
            tc.tile_pool(name="persist", bufs=1) as pp,
            tc.tile_pool(name="work", bufs=2) as wp,
            tc.tile_pool(name="cpsum", bufs=2, space="PSUM") as cpool,
            tc.tile_pool(name="spsum", bufs=2, space="PSUM") as spool,
        ):
            # ---------- persistent tiles ----------
            x_sb = pp.tile([128, NGRP * 1156], mdt, tag="x_sb")
            actA = pp.tile([128, GUARD + NGRP * S], mdt, tag="actA")
            actB = pp.tile([128, GUARD + NGRP * S], mdt, tag="actB")
            actT = pp.tile([128, 8 * 32 * SUB], mdt, tag="actT")  # [c8][ch][img]
            fc_sb = pp.tile([128, 32 * 8 * 10], mdt, tag="fc_sb")  # [ch][c8][o]
            bias_sb = pp.tile([SUB, 10], f32, tag="bias")
            w0_bd = pp.tile([128, 128], mdt, tag="w0bd")
            wbd = {(L, t): pp.tile([128, 128], mdt, tag=f"w{L}_{t}",
                                   name=f"w{L}_{t}")
                   for L in (1, 2, 3) for t in range(9)}
            ones_bd = pp.tile([128, 128], f32, tag="ones")
            ident = pp.tile([128, 128], f32, tag="ident")
            iota_a = pp.tile([128, 128], mybir.dt.int32, tag="iota_a")
            iota_b = pp.tile([128, 128], mybir.dt.int32, tag="iota_b")

            # ---------- init: zeros, weights, identity ----------
            def ms0(ap):
                nc.vector.memset(ap.bitcast(f32) if MM_DT != "float32"
                                 else ap, 0.0)
            ms0(x_sb[:])
            ms0(actA[:])
            ms0(actB[:])
            ms0(w0_bd[:])
            nc.vector.memset(ones_bd[:], 0.0)
            for j in range(4):
                nc.vector.memset(ones_bd[32*j:32*j+32, 32*j:32*j+32], 1.0)

            # identity: iota col (per-row 0..127), iota row (partition idx)
            nc.gpsimd.iota(iota_a[:], pattern=[[1, 128]], base=0,
                           channel_multiplier=0)
            nc.gpsimd.iota(iota_b[:], pattern=[[0, 128]], base=0,
                           channel_multiplier=1)
            nc.vector.tensor_tensor(ident[:], iota_a[:], iota_b[:],
                                    ALU.is_equal)

            # conv_w0 -> w0_bd[32j + 3t + c, 32j + o]
            # dram conv_w0 [o, c, dy, dx]; per j one DMA:
            # out dims [t 9 (p-step 3), c 3 (p-step 1)][free o 32]
            w0src = w0_d[:].rearrange("o c dy dx -> (dy dx) c o")
            for j in range(4):
                for t in range(9):
                    nc.sync.dma_start(
                        out=w0_bd[32*j+3*t:32*j+3*t+3, 32*j:32*j+32],
                        in_=dmt(w0src[t]))
            # conv_w{1..3} -> wbd[(L,t)][32j+ci, 32j+co] = w[co, ci, dy, dx]
            for L in (1, 2, 3):
                wsrc = w_d[L][:].rearrange("co ci dy dx -> (dy dx) ci co")
                for t in range(9):
                    ms0(wbd[(L, t)][:])
                    for j in range(4):
                        nc.sync.dma_start(
                            out=wbd[(L, t)][32*j:32*j+32, 32*j:32*j+32],
                            in_=dmt(wsrc[t]))

            # fc_w -> fc_sb[p, (ch, c8, o)] = fc_w[o, ch*1024 + c8*128 + p]
            fcv = fc_sb[:].rearrange("p (ch c8 o) -> p ch c8 o", ch=32, c8=8)
            fsrc = fcw_d[:].rearrange("o (ch c8 p) -> ch c8 p o", ch=32, c8=8)
            for ch in range(32):
                for c8 in range(8):
                    nc.sync.dma_start(out=fcv[:, ch, c8], in_=dmt(fsrc[ch, c8]))
            # bias replicated across SUB partitions
            for i in range(SUB):
                nc.sync.dma_start(out=bias_sb[i:i+1, :], in_=fcb_d[None, :])

            # ---------- main loop ----------
            xr = x_d[:].rearrange("(s g il) c h w -> s il g c h w",
                                  s=NSUB, g=NGRP)
            x_sbv = x_sb[:].rearrange("(il pc) (g q) -> il pc g q",
                                      il=4, g=NGRP)

            import contextlib
            rep_ctx = tc.For_i(0, REPEAT, 1) if REPEAT else \
                contextlib.nullcontext()
            with rep_ctx:
              for s in range(NSUB):
                # L0 im2col: 27 DMAs, each tap written pixel-shifted
                xq = x_sbv.rearrange("il pc g (yy xx) -> il pc g yy xx", yy=34)
                for t, (dy, dx) in enumerate(TAPS):
                    for c in range(3):
                        for g in range(NGRP):
                            nc.sync.dma_start(
                                out=xq[:, 3*t + c, g,
                                       1-dy:33-dy, 1-dx:33-dx],
                                in_=dmt(xr[s, :, g, c, :, :]))

                bufs = [actA, actB]
                aTv = actT[:].rearrange("p (c8 ch i) -> p c8 ch i",
                                        c8=8, ch=32)
                for L in range(4):
                    dst = bufs[L % 2]
                    src = bufs[(L - 1) % 2]
                    for g in range(NGRP):
                        psum = cpool.tile([128, 1536], f32, tag="conv")
                        for ci, (r0, r1) in enumerate(CHUNKS):
                            n = (r1 - r0) * 34
                            po = psum[:, 512*ci: 512*ci + n]
                            if L == 0:
                                rhs = x_sb[:, g*1156 + r0*34:
                                           g*1156 + r0*34 + n]
                                nc.tensor.matmul(po, w0_bd[:], rhs,
                                                 start=True, stop=True)
                            else:
                                for t in range(9):
                                    dy, dx = TAPS[t]
                                    o = GUARD + g*S + r0*34 + dy*34 + dx
                                    rhs = src[:, o:o+n]
                                    nc.tensor.matmul(
                                        po, wbd[(L, t)][:], rhs,
                                        start=(t == 0), stop=(t == 8))

                        # chunk ci as [p, y, x] rows r0..r1 of 34 pixels
                        def chunk_rows(ci):
                            r0, r1 = CHUNKS[ci]
                            return psum[:, 512*ci: 512*ci + (r1-r0)*34] \
                                .rearrange("p (y x) -> p y x", x=34)

                        # ---- SRePro ----
                        sq = wp.tile([128, 3], f32, tag="sq")
                        scratch = wp.tile([128, 408], f32, tag="scr")
                        for ci, (r0, r1) in enumerate(CHUNKS):
                            y0, y1 = max(r0, 1), min(r1, 33)
                            iv = chunk_rows(ci)[:, y0-r0:y1-r0, 1:33]
                            nc.scalar.activation(
                                scratch[:, :(y1-y0)*32].rearrange(
                                    "p (y x) -> p y x", x=32),
                                iv, AF.Square, accum_out=sq[:, ci:ci+1])
                        sqt = wp.tile([128, 1], f32, tag="sqt")
                        nc.vector.tensor_reduce(
                            sqt[:], sq[:], axis=mybir.AxisListType.X,
                            op=ALU.add)
                        nrm = spool.tile([128, 1], f32, tag="small")
                        nc.tensor.matmul(nrm[:], ones_bd[:], sqt[:],
                                         start=True, stop=True)
                        scl = wp.tile([128, 1], f32, tag="scl")
                        scl2 = wp.tile([128, 1], f32, tag="scl2")
                        nc.vector.tensor_scalar(scl[:], nrm[:], 0.5, 1.0,
                                                ALU.mult, ALU.add)
                        nc.vector.reciprocal(scl2[:], scl[:])

                        # ---- scaled copy out of PSUM ----
                        if L < 3:
                            base = GUARD + g*S
                            dv = dst[:, base: base + 1156].rearrange(
                                "p (y x) -> p y x", x=34)
                            for ci, (r0, r1) in enumerate(CHUNKS):
                                y0, y1 = max(r0, 1), min(r1, 33)
                                iv = chunk_rows(ci)[:, y0-r0:y1-r0, 1:33]
                                nc.vector.tensor_scalar(
                                    dv[:, y0:y1, 1:33], iv, scl2[:],
                                    None, ALU.mult)
                        else:
                            comp = wp.tile([128, 1024], f32, tag="comp")
                            cv = comp[:].rearrange("p (y x) -> p y x", x=32)
                            for ci, (r0, r1) in enumerate(CHUNKS):
                                y0, y1 = max(r0, 1), min(r1, 33)
                                iv = chunk_rows(ci)[:, y0-r0:y1-r0, 1:33]
                                nc.vector.tensor_scalar(
                                    cv[:, y0-1:y1-1, :], iv, scl2[:],
                                    None, ALU.mult)
                            # transpose compact -> actT right away
                            for c8 in range(8):
                                pT = spool.tile([128, 128], f32, tag="small")
                                nc.tensor.transpose(
                                    pT[:], comp[:, 128*c8:128*(c8+1)],
                                    ident[:])
                                # pT[pix, il*32+ch] -> actT[pix,c8,ch,g4+il]
                                nc.vector.tensor_copy(
                                    aTv[:, c8, :, 4*g:4*g+4],
                                    pT[:].rearrange("p (il ch) -> p ch il",
                                                    il=4))

                # ---- FC ----
                yp = spool.tile([SUB, 10], f32, tag="small")
                nmm = 0
                for c8 in range(8):
                    for ch in range(32):
                        nc.tensor.matmul(
                            yp[:], aTv[:, c8, ch, :],
                            fcv[:, ch, c8, :],
                            start=(nmm == 0), stop=(nmm == 255))
                        nmm += 1
                y_sb = wp.tile([SUB, 10], f32, tag="ysb")
                nc.vector.tensor_tensor(y_sb[:], yp[:], bias_sb[:], ALU.add)
                nc.sync.dma_start(out=y_d[SUB*s:SUB*(s+1), :], in_=y_sb[:])

    nc.compile()
    return nc


_NC_CACHE = None


def kernel(**inputs):
    global _NC_CACHE
    from concourse.bass_utils import run_bass_kernel_spmd

    if _NC_CACHE is None:
        _NC_CACHE = build_bass()
    nc = _NC_CACHE

    x = np.ascontiguousarray(inputs["x"], np.float32)
    shared = {k: np.ascontiguousarray(np.asarray(inputs[k]), np.float32)
              for k in ("conv_w0", "conv_w1", "conv_w2", "conv_w3",
                        "fc_w", "fc_b")}
    in_maps = [
        {"x": x[i*B_PER_CORE:(i+1)*B_PER_CORE], **shared} for i in range(CORES)
    ]
    res = run_bass_kernel_spmd(nc, in_maps, core_ids=list(range(CORES)))
    return np.concatenate([r["y"] for r in res.results], axis=0)
